# revision 21
# baseline (speedup 1.0000x reference)
"""Multi-head causal attention with RoPE on 8 TRN2 NeuronCores.

Problem: B=2, T=2048, D=1024, H=16 heads (dh=64), fp32 I/O.
  q/k/v = x @ w{q,k,v}.T ; RoPE(q,k) ; causal softmax((q k^T)/sqrt(dh)) @ v ;
  out = concat_heads @ wo.T

Sharding (8 cores): head-parallel compute, token-striped output. Core c owns
heads {2c, 2c+1} for both batches; four AllToAll collectives redistribute
attention outputs so core c ends up with all 1024 features for its four
128-token chunks {c, 8+c, 16+c, 24+c}; it then applies the output projection.

v2 design notes (driven by the baseline's perfetto profile):
 - Engine-queue discipline: ACT runs ONLY the exp stream (the o65/vtt/fo
   copies moved to DVE, causal-mask muls to GpSimd, weight DMAs to the
   GpSimd DGE queue). The baseline's ACT queue carried 43us of copies+DMA
   which head-blocked the PE whenever it waited for an S-psum bank drain.
 - Chunk-granular software pipeline: projection / output-projection matmuls
   are interleaved 1-2 per attention chunk from a FIFO filler queue, so the
   PE always has independent work while ACT chews exp (exp ~1.15us/chunk vs
   ~0.8us of attention matmul per chunk). This also keeps the PE out of the
   HAM K=4/8 half-clock re-throttle state (idle >3.4us triggers it).
 - All input DMAs are flat [128, N] copies: the host pre-arranges xT and the
   weights into the exact SBUF layout (the baseline's (k p) c -> p k c
   rearranged DMAs issued 256B-element descriptor storms; first matmul was
   at t=16us, now ~9us).
 - reciprocal -> reciprocal_approx_fast (13.3us -> ~2.7us of DVE).
 - A2A staging is one 4D-AP DMA per (j,h) unit instead of 4 2D DMAs; a2a
   output is loaded as 8 flat [128,128] chunks consumed directly as lhsT.
 - Tail: the last a2a is covered by held-back final-projection matmuls for
   groups 1-2 so the PE stays busy (and warm) during the collective.

Layout notes (unchanged from baseline):
 - Host pre-transposes x -> xT [D, B*T] so projections produce q^T/k^T
   directly; RoPE pairs de-interleaved on the host via wq/wk row permutation
   (16 re rows then 16 im rows per 32-row quadrant); partner swap is a DVE
   stream_shuffle.
 - A ones column is appended to each V chunk so the softmax denominator
   falls out of the PV matmul (row 64 of the accumulator).
 - The two heads' K=64 QK matmuls use PE row-groups 0-63 / 64-127 and run
   concurrently in the systolic array.
 - Softmax skips the running-max: |scores|/8 < ~6 for unit-variance inputs.
"""

from collections import deque

import numpy as np
import ml_dtypes

import concourse.bacc as bacc
import concourse.tile as tile
import concourse.mybir as mybir
from concourse import bass_utils

BF16 = mybir.dt.bfloat16
F32 = mybir.dt.float32
AF = mybir.ActivationFunctionType

NCORES = 8
B, T, D, H = 2, 2048, 1024, 16
DH = D // H          # 64
HPC = H // NCORES    # 2 heads per core
FPC = DH * HPC       # 128 features per core
TOK = B * T          # 4096
TPC = TOK // NCORES  # 512 tokens per core (output shard)
KC = D // 128        # 8 contraction chunks
NT = T // 512        # 4 query tiles of 512 per batch
VG = 256             # cols per v-group: [v_h0(64) | 1 | pad | v_h1(64) | 1 | pad]

_COMPILED = None


def _build():
    nc = bacc.Bacc("TRN2", target_bir_lowering=False, debug=False, num_devices=NCORES)

    # All inputs are host-prearranged into their exact SBUF layouts (flat DMAs).
    xT_d = nc.dram_tensor("xTf", [128, KC * TOK], BF16, kind="ExternalInput")
    wq_d = nc.dram_tensor("wqf", [128, KC * FPC], BF16, kind="ExternalInput")
    wk_d = nc.dram_tensor("wkf", [128, KC * FPC], BF16, kind="ExternalInput")
    wv_d = nc.dram_tensor("wvf", [128, KC * FPC], BF16, kind="ExternalInput")
    wo_d = nc.dram_tensor("wof", [128, KC * D], BF16, kind="ExternalInput")
    C_d = nc.dram_tensor("cosC", [128, T], BF16, kind="ExternalInput")
    S_d = nc.dram_tensor("sinS", [128, T], BF16, kind="ExternalInput")
    mask2_d = nc.dram_tensor("mask2", [128, 256], BF16, kind="ExternalInput")
    id_d = nc.dram_tensor("ident", [128, 128], BF16, kind="ExternalInput")
    sel_d = nc.dram_tensor("sel", [4, 4 * DH], BF16, kind="ExternalInput")
    out_d = nc.dram_tensor("out", [TPC, D], F32, kind="ExternalOutput")

    swap16 = list(range(16, 32)) + list(range(16))

    with tile.TileContext(nc) as tc:
        with (
            tc.tile_pool(name="sb", bufs=1) as sb,
            tc.tile_pool(name="ps", bufs=1, space="PSUM") as ps,
            tc.tile_pool(name="dram", bufs=1, space="DRAM") as dram,
        ):
            # ---- persistent SBUF residents ----
            wq_sb = sb.tile([128, KC * FPC], BF16)
            wk_sb = sb.tile([128, KC * FPC], BF16)
            wv_sb = sb.tile([128, KC * FPC], BF16)
            wo_sb = sb.tile([128, KC * D], BF16)
            C_sb = sb.tile([128, T], BF16)
            S_sb = sb.tile([128, T], BF16)
            mask2_sb = sb.tile([128, 256], BF16)
            id_sb = sb.tile([128, 128], BF16)
            sel_sb = sb.tile([4, 4 * DH], BF16)
            xT_sb = sb.tile([128, KC * TOK], BF16)
            qrot_sb = sb.tile([128, TOK], BF16)
            krot_sb = sb.tile([128, TOK], BF16)
            v1_sb = sb.tile([128, B * (T // 128) * VG], BF16)

            # ones column (softmax denominator) in each 128-col v slot
            nc.gpsimd.memset(
                v1_sb[:].rearrange("p (g c) -> p g c", c=128)[:, :, 64:65], 1.0
            )
            # warm the exp table set while DMAs stream (first real exp is ~25us in)
            tblw = sb.tile([1, 1], F32)
            nc.scalar.activation(tblw[:], v1_sb[0:1, 64:65], AF.Exp)

            # ---- input DMAs, all flat copies on HWDGE queues (gpsimd DMAs
            # are software-DGE at ~24GB/s - never put loads there).
            # sync: wq + first xT block first (gates the first matmul), then
            # the rest of the stream. vector: rope/mask constants (DVE is
            # scalar queue is idle until the first exp at ~25us).
            # fine-grained first chunks of wq (scalar queue) and xT block
            # (0,0) (sync queue), overlapping: the DMA path is cold at kernel
            # start, so small pieces land much earlier and the first
            # projection matmuls start sooner
            nc.scalar.dma_start(wq_sb[:, :FPC], wq_d[:, :FPC])
            nc.sync.dma_start(xT_sb[:, :512], xT_d[:, :512])
            nc.scalar.dma_start(wq_sb[:, FPC : 2 * FPC], wq_d[:, FPC : 2 * FPC])
            nc.sync.dma_start(xT_sb[:, 512:1024], xT_d[:, 512:1024])
            nc.scalar.dma_start(wq_sb[:, 2 * FPC : 4 * FPC], wq_d[:, 2 * FPC : 4 * FPC])
            nc.sync.dma_start(xT_sb[:, 1024:2048], xT_d[:, 1024:2048])
            nc.scalar.dma_start(wq_sb[:, 4 * FPC :], wq_d[:, 4 * FPC :])
            nc.sync.dma_start(xT_sb[:, 2048 : KC * 512], xT_d[:, 2048 : KC * 512])

            def xt_load(b, n):
                blk = (NT * b + n) * (KC * 512)
                nc.sync.dma_start(
                    xT_sb[:, blk : blk + KC * 512], xT_d[:, blk : blk + KC * 512]
                )
            nc.sync.dma_start(wk_sb[:], wk_d[:])
            nc.sync.dma_start(wv_sb[:], wv_d[:])
            nc.scalar.dma_start(C_sb[:], C_d[:])
            nc.scalar.dma_start(S_sb[:], S_d[:])
            nc.scalar.dma_start(id_sb[:], id_d[:])
            nc.scalar.dma_start(mask2_sb[:], mask2_d[:])
            nc.scalar.dma_start(sel_sb[:], sel_d[:])
            xt_load(0, 1)
            xt_load(0, 2)
            xt_load(0, 3)
            nc.sync.dma_start(wo_sb[:], wo_d[:])
            for n in range(NT):
                xt_load(1, n)

            # 4 AllToAll groups: group g carries global token chunks 8g+o to rank o
            a2a_in = [dram.tile([D, 128], BF16, name=f"a2ain{g}") for g in range(4)]
            a2a_out = [dram.tile([D, 128], BF16, name=f"a2aout{g}") for g in range(4)]

            # ================= filler queue machinery =================
            FQ = deque()          # items: (cost, label, fn)
            emitted = set()

            def push(units):
                FQ.extend(units)

            def _pop_one():
                cost, label, fn = FQ.popleft()
                fn()
                if label is not None:
                    emitted.add(label)
                return cost

            def drain(budget):
                while FQ and budget > 0:
                    budget -= _pop_one()
                # emit trailing zero-cost items (rope chains etc.) promptly
                while FQ and FQ[0][0] == 0:
                    _pop_one()

            def drain_until(lbl):
                if lbl in emitted:
                    return
                while FQ:
                    _, label, _ = FQ[0]
                    _pop_one()
                    if label == lbl:
                        return
                raise RuntimeError(f"label {lbl} not found in filler queue")

            def drain_all():
                while FQ:
                    _pop_one()

            # ================= projection tiles =================
            def ptile_units(b, n):
                """3 projection chains (q,k,v) + rope + V transpose for token
                block (b,n), as interleavable units."""
                st = {}
                blk = (NT * b + n) * (KC * 512)

                def mm(which, w_sb, kc):
                    def f():
                        if kc == 0:
                            st[which] = ps.tile(
                                [128, 512], F32, tag="proj", bufs=2,
                                name=f"pp{which}{b}{n}",
                            )
                        nc.tensor.matmul(
                            st[which][:],
                            w_sb[:, kc * FPC : (kc + 1) * FPC],
                            xT_sb[:, blk + 512 * kc : blk + 512 * kc + 512],
                            start=(kc == 0),
                            stop=(kc == KC - 1),
                        )
                    return (1.0, None, f)

                def rope(which, dst_sb):
                    def f():
                        pp = st[which]
                        swp = sb.tile([128, 512], F32, tag="swp", bufs=3,
                                      name=f"swp{which}{b}{n}")
                        nc.vector.stream_shuffle(swp[:], pp[:], swap16)
                        t1 = sb.tile([128, 512], BF16, tag="t1", bufs=3,
                                     name=f"t1{which}{b}{n}")
                        nc.vector.tensor_mul(t1[:], pp[:], C_sb[:, 512 * n : 512 * n + 512])
                        t2 = sb.tile([128, 512], BF16, tag="t2", bufs=3,
                                     name=f"t2{which}{b}{n}")
                        nc.vector.tensor_mul(t2[:], swp[:], S_sb[:, 512 * n : 512 * n + 512])
                        nc.vector.tensor_add(
                            dst_sb[:, b * T + 512 * n : b * T + 512 * n + 512],
                            t1[:], t2[:],
                        )
                    return (0.0, None, f)

                def vtt_copy():
                    def f():
                        vtt = sb.tile([128, 512], BF16, tag="vtt", bufs=2,
                                      name=f"vtt{b}{n}")
                        nc.vector.tensor_copy(vtt[:], st["v"][:])
                        st["vtt"] = vtt
                    return (0.0, None, f)

                def tp_i(i):
                    def f():
                        g = VG * ((T // 128) * b + 4 * n + i)
                        tp = ps.tile([128, 128], BF16, tag="proj", bufs=2,
                                     name=f"tp{b}{n}{i}")
                        nc.tensor.matmul(
                            tp[:],
                            st["vtt"][:, 128 * i : 128 * i + 128],
                            id_sb[:],
                            is_transpose=True,
                            start=True,
                            stop=True,
                        )
                        for h in range(2):
                            nc.vector.tensor_copy(
                                v1_sb[:, g + 128 * h : g + 128 * h + 64],
                                tp[:, 64 * h : 64 * h + 64],
                            )
                    return (0.5, None, f)

                units = []
                for kc in range(KC):
                    units.append(mm("q", wq_sb, kc))
                units.append(rope("q", qrot_sb))
                for kc in range(KC):
                    units.append(mm("k", wk_sb, kc))
                units.append(rope("k", krot_sb))
                for kc in range(KC):
                    units.append(mm("v", wv_sb, kc))
                units.append(vtt_copy())
                for i in range(4):
                    units.append(tp_i(i))
                # end marker (zero-cost no-op)
                units.append((0.0, ("Pend", b, n), lambda: None))
                return units

            # ================= attention =================
            def attn_tile(b, j):
                """Both heads for (batch b, q-tile j), chunk-granular with
                filler interleave. Returns [65,512] f32 SBUF copies of the
                two O accumulators (row 64 = softmax sums)."""
                drain_until(("Pend", b, j))
                ops = [
                    ps.tile([65, 512], F32, tag="opsum", bufs=2, name=f"op{b}{h}{j}")
                    for h in range(2)
                ]
                nch = 4 * j + 4

                def qk_exp(c):
                    diag = c - 4 * j
                    lo = 128 * diag if diag >= 0 else 0
                    sp = ps.tile([128, 1024], F32, tag="spsum", bufs=2,
                                 name=f"sp{b}{j}{c}")
                    spv = sp[:].rearrange("p (h t) -> p h t", h=2)
                    for h in range(2):
                        nc.tensor.matmul(
                            sp[:, 512 * h + lo : 512 * h + 512],
                            krot_sb[64 * h : 64 * h + 64,
                                    b * T + 128 * c : b * T + 128 * c + 128],
                            qrot_sb[64 * h : 64 * h + 64,
                                    b * T + 512 * j + lo : b * T + 512 * j + 512],
                            start=True,
                            stop=True,
                        )
                    pt = sb.tile([128, 1024], BF16, tag="pt", bufs=4,
                                 name=f"pt{b}{j}{c}")
                    ptv = pt[:].rearrange("p (h t) -> p h t", h=2)
                    nc.scalar.activation(
                        ptv[:, :, lo:512], spv[:, :, lo:512], AF.Exp, scale=0.125
                    )
                    if diag >= 0:
                        # zero the upper triangle post-exp (GpSimd: keeps DVE
                        # free for rope and keeps ACT pure-exp)
                        nc.gpsimd.tensor_mul(
                            ptv[:, :, lo : lo + 128], ptv[:, :, lo : lo + 128],
                            mask2_sb[:].rearrange("p (h t) -> p h t", h=2),
                        )
                    return pt

                def pv(c, pt):
                    diag = c - 4 * j
                    lo = 128 * diag if diag >= 0 else 0
                    g = VG * ((T // 128) * b + c)
                    for h in range(2):
                        nc.tensor.matmul(
                            ops[h][:, lo:512],
                            v1_sb[:, g + 128 * h : g + 128 * h + 65],
                            pt[:, 512 * h + lo : 512 * h + 512],
                            start=(c == 0),
                            stop=(c == nch - 1),
                        )

                prev = None
                for c in range(nch):
                    cur = qk_exp(c)
                    if prev is not None:
                        pv(c - 1, prev)
                    # interleave filler so the PE never head-blocks on exp
                    drain(2.5 if c < 4 * j else 1.5)
                    prev = cur
                pv(nch - 1, prev)
                o65s = []
                for h in range(2):
                    o65 = sb.tile([65, 512], F32, tag="o65", bufs=4,
                                  name=f"o65{b}{h}{j}")
                    nc.vector.tensor_copy(o65[:], ops[h][:])
                    o65s.append(o65)
                return o65s

            # ================= epilogue + collective =================
            at_tiles = {}

            def epi_pre(b, jpair, o65_by_j, sums_on_sync=False):
                """Per-pair normalization prefix: gather the 4 sum rows, one
                fast reciprocal, cast to bf16. All DVE/DMA - off the PE path.
                Mid-kernel pairs gather on the scalar queue (the sync queue
                carries at-loads whose collective waits would head-block
                these); the tail pair uses sync (no exp drain to wait out)."""
                units_meta = []
                for j in jpair:
                    for h in range(2):
                        units_meta.append((j, h, o65_by_j[j][h]))
                q = nc.sync if sums_on_sync else nc.scalar
                sums = sb.tile([4, 512], F32, tag="sums", bufs=2,
                               name=f"sums{b}{jpair[0]}")
                for r, (j, h, o65) in enumerate(units_meta):
                    q.dma_start(sums[r : r + 1, :], o65[64:65, :])
                rec4 = sb.tile([4, 512], F32, tag="rec4", bufs=2,
                               name=f"rec4{b}{jpair[0]}")
                nc.vector.reciprocal_approx_fast(out=rec4[:], in_=sums[:])
                recb4 = sb.tile([4, 512], BF16, tag="recb4", bufs=2,
                                name=f"recb4{b}{jpair[0]}")
                nc.vector.tensor_copy(recb4[:], rec4[:])
                return units_meta, recb4

            def epi_units(b, jpair, g, units_meta, recb4):
                """sel-matmul + normalize + stage, one unit per (j,h); then the
                collective trigger and the a2a-output loads."""
                units = []

                def unit(r, j, h, o65):
                    def f():
                        bps = ps.tile([64, 512], F32, tag="proj", bufs=2,
                                      name=f"bps{b}{j}{h}")
                        nc.tensor.matmul(
                            bps[:], sel_sb[:, DH * r : DH * r + DH], recb4[:],
                            start=True, stop=True,
                        )
                        onr = sb.tile([64, 512], BF16, tag="onr", bufs=4,
                                      name=f"onr{b}{j}{h}")
                        nc.vector.tensor_mul(onr[:], o65[0:64, :], bps[:])
                        # one 3D-AP staging DMA (partition dim first): 4
                        # token-chunks -> consecutive ranks o0..o0+3 of group g
                        o0 = 4 * (j % 2)
                        dst = a2a_in[g][:].rearrange(
                            "(o hh p) t -> hh p o t", hh=2, p=64
                        )[h, :, o0 : o0 + 4]
                        nc.scalar.dma_start(
                            dst, onr[:].rearrange("p (i t) -> p i t", i=4)
                        )
                    return (1.0, None, f)

                for r, (j, h, o65) in enumerate(units_meta):
                    units.append(unit(r, j, h, o65))

                def cc():
                    nc.gpsimd.collective_compute(
                        "AllToAll",
                        mybir.AluOpType.bypass,
                        replica_groups=[list(range(NCORES))],
                        ins=[a2a_in[g].opt()],
                        outs=[a2a_out[g].opt()],
                    )
                units.append((0.0, ("cc", g), cc))
                return units

            def load_at(g, nsplit=2):
                """Must be emitted AFTER collective g completes but BEFORE
                collective g+1's trigger pops - otherwise the DMA's wait
                resolves against the later collective and head-blocks the
                sync queue for tens of us. nsplit=8 (tail) lets final-matmul
                kc start as soon as chunk kc lands."""
                at = sb.tile([128, KC * 128], BF16, tag="at", bufs=4, name=f"at{g}")
                step = KC // nsplit
                for i in range(nsplit):
                    nc.sync.dma_start(
                        at[:].rearrange("p (o t) -> p o t", o=KC)[
                            :, step * i : step * i + step
                        ],
                        a2a_out[g][:].rearrange("(o p) t -> p o t", p=128)[
                            :, step * i : step * i + step
                        ],
                    )
                at_tiles[g] = at

            # ================= output projection =================
            def final_units(g, nhs=(0, 1)):
                units = []
                st = {}

                def mm(nh, kc):
                    def f():
                        if kc == 0:
                            st[nh] = ps.tile([128, 512], F32, tag="proj", bufs=2,
                                             name=f"fp{g}{nh}")
                        nc.tensor.matmul(
                            st[nh][:],
                            at_tiles[g][:, 128 * kc : 128 * kc + 128],
                            wo_sb[:, kc * D + 512 * nh : kc * D + 512 * nh + 512],
                            start=(kc == 0),
                            stop=(kc == KC - 1),
                        )
                    return (1.0, None, f)

                def store(nh):
                    def f():
                        fo = sb.tile([128, 512], F32, tag="fo", bufs=2,
                                     name=f"fo{g}{nh}")
                        nc.vector.tensor_copy(fo[:], st[nh][:])
                        nc.sync.dma_start(
                            out_d[128 * g : 128 * g + 128,
                                  512 * nh : 512 * nh + 512],
                            fo[:],
                        )
                    return (0.0, None, f)

                for nh in nhs:
                    for kc in range(KC):
                        units.append(mm(nh, kc))
                    units.append(store(nh))
                return units

            # ================= schedule =================
            o0, o1 = {}, {}
            push(ptile_units(0, 0))
            push(ptile_units(0, 1))
            drain_all()                       # dense startup
            push(ptile_units(0, 2))
            o0[0] = attn_tile(0, 0)
            push(ptile_units(0, 3))
            o0[1] = attn_tile(0, 1)
            m, r = epi_pre(0, (0, 1), o0)
            push(epi_units(0, (0, 1), 0, m, r))
            push(ptile_units(1, 0))
            o0[2] = attn_tile(0, 2)
            push(ptile_units(1, 1))
            o0[3] = attn_tile(0, 3)
            m, r = epi_pre(0, (2, 3), o0)
            load_at(0)                        # after cc0 done, before cc1 trigger
            push(epi_units(0, (2, 3), 1, m, r))
            push(ptile_units(1, 2))
            o1[0] = attn_tile(1, 0)
            o1[1] = attn_tile(1, 1)
            m, r = epi_pre(1, (0, 1), o1)
            load_at(1)                        # after cc1 done, before cc2 trigger
            push(epi_units(1, (0, 1), 2, m, r))
            push(ptile_units(1, 3))
            o1[2] = attn_tile(1, 2)
            push(final_units(0))
            o1[3] = attn_tile(1, 3)
            # last epilogue: start its DVE chain now, cover its latency with
            # leftover fillers + F1, then cover cc3's execution with F2
            m, r = epi_pre(1, (2, 3), o1, sums_on_sync=True)
            push(final_units(1))
            push(epi_units(1, (2, 3), 3, m, r))
            load_at(2)                        # after cc2 done, before cc3 trigger
            drain_all()                       # leftovers + F1, then sel/stage + cc3
            push(final_units(2))
            drain_all()
            load_at(3, nsplit=8)
            push(final_units(3))
            drain_all()

    nc.compile()
    return nc


def _get_compiled():
    global _COMPILED
    if _COMPILED is None:
        _COMPILED = _build()
    return _COMPILED


def _kc_flat(a):
    """[D, N] -> [128, KC*N] with row-chunk kc at column block kc."""
    Dn, N = a.shape
    assert Dn == KC * 128
    return np.ascontiguousarray(
        a.reshape(KC, 128, N).transpose(1, 0, 2).reshape(128, KC * N)
    )


def _prep_in_maps(embedding_word, wq, wk, wv, wo):
    bf = ml_dtypes.bfloat16
    x = np.asarray(embedding_word, np.float32).reshape(TOK, D)

    # xT in the exact SBUF layout: token-block (b,n) major, kc-chunk inside
    x5 = x.reshape(B, NT, 512, KC, 128)            # [b, n, t, kc, p]
    xTf = np.ascontiguousarray(
        x5.transpose(4, 0, 1, 3, 2).reshape(128, KC * TOK)
    ).astype(bf)

    wof = _kc_flat(np.ascontiguousarray(np.asarray(wo, np.float32).T)).astype(bf)

    # within-head row permutation: 16 re rows then 16 im rows per 32-row quadrant
    perm64 = [
        (2 * (16 * q + r) if r < 16 else 2 * (16 * q + (r - 16)) + 1)
        for q in range(2)
        for r in range(32)
    ]
    perm64 = np.asarray(perm64)

    freqs = 1.0 / (10000.0 ** (np.arange(0, DH, 2, dtype=np.float64) / DH))  # [32]
    ang = np.arange(T, dtype=np.float64)[:, None] * freqs[None, :]  # [T, 32]
    cos_t, sin_t = np.cos(ang), np.sin(ang)
    rows = np.arange(128)
    wh = rows % 64
    qd = wh // 32
    r32 = wh % 32
    dmap = 16 * qd + (r32 % 16)
    sign = np.where(r32 < 16, -1.0, 1.0)
    C = np.ascontiguousarray(cos_t[:, dmap].T).astype(bf)  # [128, T]
    S = np.ascontiguousarray((sin_t[:, dmap] * sign[None, :]).T).astype(bf)

    rr = np.arange(128)[:, None]
    cc = np.arange(128)[None, :]
    mask = np.where(cc >= rr, 1.0, 0.0).astype(np.float32)
    mask2 = np.ascontiguousarray(np.concatenate([mask, mask], axis=1)).astype(bf)
    ident = np.eye(128, dtype=np.float32).astype(bf)
    sel = np.zeros((4, 4 * DH), np.float32)
    for r in range(4):
        sel[r, DH * r : DH * r + DH] = 1.0
    sel = sel.astype(bf)

    wqf = np.asarray(wq, np.float32)
    wkf = np.asarray(wk, np.float32)
    wvf = np.asarray(wv, np.float32)

    in_maps = []
    for c in range(NCORES):
        rows_c = slice(FPC * c, FPC * c + FPC)
        wq_c = wqf[rows_c].reshape(HPC, DH, D)[:, perm64, :].reshape(FPC, D)
        wk_c = wkf[rows_c].reshape(HPC, DH, D)[:, perm64, :].reshape(FPC, D)
        wv_c = wvf[rows_c]
        in_maps.append(
            {
                "xTf": xTf,
                "wqf": _kc_flat(np.ascontiguousarray(wq_c.T)).astype(bf),
                "wkf": _kc_flat(np.ascontiguousarray(wk_c.T)).astype(bf),
                "wvf": _kc_flat(np.ascontiguousarray(wv_c.T)).astype(bf),
                "wof": wof,
                "cosC": C,
                "sinS": S,
                "mask2": mask2,
                "ident": ident,
                "sel": sel,
            }
        )
    return in_maps


def _unshard(core_outs):
    """core_outs[c] is [TPC, D] covering token chunks {c, 8+c, 16+c, 24+c}
    (row-blocks g=0..3). Interleave back to [B, T, D]."""
    a = np.stack(core_outs, axis=0)  # [8, TPC, D]
    a = a.reshape(NCORES, 4, 128, D).transpose(1, 0, 2, 3).reshape(TOK, D)
    return np.ascontiguousarray(a.reshape(B, T, D).astype(np.float32))


def kernel(embedding_word, wq, wk, wv, wo):
    nc = _get_compiled()
    in_maps = _prep_in_maps(embedding_word, wq, wk, wv, wo)
    res = bass_utils.run_bass_kernel_spmd(nc, in_maps, core_ids=list(range(NCORES)))
    return _unshard([res.results[c]["out"] for c in range(NCORES)])


# revision 22
# speedup vs baseline: 1.0474x; 1.0474x over previous
"""Multi-head causal attention with RoPE on 8 TRN2 NeuronCores.

Problem: B=2, T=2048, D=1024, H=16 heads (dh=64), fp32 I/O.
  q/k/v = x @ w{q,k,v}.T ; RoPE(q,k) ; causal softmax((q k^T)/sqrt(dh)) @ v ;
  out = concat_heads @ wo.T

Sharding (8 cores): head-parallel compute, token-striped output. Core c owns
heads {2c, 2c+1} for both batches; four AllToAll collectives redistribute
attention outputs so core c ends up with all 1024 features for its four
128-token chunks {c, 8+c, 16+c, 24+c}; it then applies the output projection.

v2 design notes (driven by the baseline's perfetto profile):
 - Engine-queue discipline: ACT runs ONLY the exp stream (the o65/vtt/fo
   copies moved to DVE, causal-mask muls to GpSimd, weight DMAs to the
   GpSimd DGE queue). The baseline's ACT queue carried 43us of copies+DMA
   which head-blocked the PE whenever it waited for an S-psum bank drain.
 - Chunk-granular software pipeline: projection / output-projection matmuls
   are interleaved 1-2 per attention chunk from a FIFO filler queue, so the
   PE always has independent work while ACT chews exp (exp ~1.15us/chunk vs
   ~0.8us of attention matmul per chunk). This also keeps the PE out of the
   HAM K=4/8 half-clock re-throttle state (idle >3.4us triggers it).
 - All input DMAs are flat [128, N] copies: the host pre-arranges xT and the
   weights into the exact SBUF layout (the baseline's (k p) c -> p k c
   rearranged DMAs issued 256B-element descriptor storms; first matmul was
   at t=16us, now ~9us).
 - reciprocal -> reciprocal_approx_fast (13.3us -> ~2.7us of DVE).
 - A2A staging is one 4D-AP DMA per (j,h) unit instead of 4 2D DMAs; a2a
   output is loaded as 8 flat [128,128] chunks consumed directly as lhsT.
 - Tail: the last a2a is covered by held-back final-projection matmuls for
   groups 1-2 so the PE stays busy (and warm) during the collective.

Layout notes (unchanged from baseline):
 - Host pre-transposes x -> xT [D, B*T] so projections produce q^T/k^T
   directly; RoPE pairs de-interleaved on the host via wq/wk row permutation
   (16 re rows then 16 im rows per 32-row quadrant); partner swap is a DVE
   stream_shuffle.
 - A ones column is appended to each V chunk so the softmax denominator
   falls out of the PV matmul (row 64 of the accumulator).
 - The two heads' K=64 QK matmuls use PE row-groups 0-63 / 64-127 and run
   concurrently in the systolic array.
 - Softmax skips the running-max: |scores|/8 < ~6 for unit-variance inputs.
"""

from collections import deque

import numpy as np
import ml_dtypes

import concourse.bacc as bacc
import concourse.tile as tile
import concourse.mybir as mybir
from concourse import bass_utils

BF16 = mybir.dt.bfloat16
F32 = mybir.dt.float32
AF = mybir.ActivationFunctionType

NCORES = 8
B, T, D, H = 2, 2048, 1024, 16
DH = D // H          # 64
HPC = H // NCORES    # 2 heads per core
FPC = DH * HPC       # 128 features per core
TOK = B * T          # 4096
TPC = TOK // NCORES  # 512 tokens per core (output shard)
KC = D // 128        # 8 contraction chunks
NT = T // 512        # 4 query tiles of 512 per batch
VG = 256             # cols per v-group: [v_h0(64) | 1 | pad | v_h1(64) | 1 | pad]

_COMPILED = None


def _build():
    nc = bacc.Bacc("TRN2", target_bir_lowering=False, debug=False, num_devices=NCORES)

    # All inputs are host-prearranged into their exact SBUF layouts (flat DMAs).
    xT_d = nc.dram_tensor("xTf", [128, KC * TOK], BF16, kind="ExternalInput")
    wq_d = nc.dram_tensor("wqf", [128, KC * FPC], BF16, kind="ExternalInput")
    wk_d = nc.dram_tensor("wkf", [128, KC * FPC], BF16, kind="ExternalInput")
    wv_d = nc.dram_tensor("wvf", [128, KC * FPC], BF16, kind="ExternalInput")
    wo_d = nc.dram_tensor("wof", [128, KC * D], BF16, kind="ExternalInput")
    C_d = nc.dram_tensor("cosC", [128, T], BF16, kind="ExternalInput")
    S_d = nc.dram_tensor("sinS", [128, T], BF16, kind="ExternalInput")
    mask2_d = nc.dram_tensor("mask2", [128, 256], BF16, kind="ExternalInput")
    id_d = nc.dram_tensor("ident", [128, 128], BF16, kind="ExternalInput")
    sel_d = nc.dram_tensor("sel", [4, 4 * DH], BF16, kind="ExternalInput")
    out_d = nc.dram_tensor("out", [TPC, D], F32, kind="ExternalOutput")

    swap16 = list(range(16, 32)) + list(range(16))

    with tile.TileContext(nc) as tc:
        with (
            tc.tile_pool(name="sb", bufs=1) as sb,
            tc.tile_pool(name="ps", bufs=1, space="PSUM") as ps,
            tc.tile_pool(name="dram", bufs=1, space="DRAM") as dram,
        ):
            # ---- persistent SBUF residents ----
            wq_sb = sb.tile([128, KC * FPC], BF16)
            wk_sb = sb.tile([128, KC * FPC], BF16)
            wv_sb = sb.tile([128, KC * FPC], BF16)
            wo_sb = sb.tile([128, KC * D], BF16)
            C_sb = sb.tile([128, T], BF16)
            S_sb = sb.tile([128, T], BF16)
            mask2_sb = sb.tile([128, 256], BF16)
            id_sb = sb.tile([128, 128], BF16)
            sel_sb = sb.tile([4, 4 * DH], BF16)
            xT_sb = sb.tile([128, KC * TOK], BF16)
            qrot_sb = sb.tile([128, TOK], BF16)
            krot_sb = sb.tile([128, TOK], BF16)
            v1_sb = sb.tile([128, B * (T // 128) * VG], BF16)

            # ones column (softmax denominator) in each 128-col v slot
            nc.gpsimd.memset(
                v1_sb[:].rearrange("p (g c) -> p g c", c=128)[:, :, 64:65], 1.0
            )
            # warm the exp table set while DMAs stream (first real exp is ~25us in)
            tblw = sb.tile([1, 1], F32)
            nc.scalar.activation(tblw[:], v1_sb[0:1, 64:65], AF.Exp)

            # ---- input DMAs, all flat copies on HWDGE queues (gpsimd DMAs
            # are software-DGE at ~24GB/s - never put loads there).
            # sync: wq + first xT block first (gates the first matmul), then
            # the rest of the stream. vector: rope/mask constants (DVE is
            # scalar queue is idle until the first exp at ~25us).
            # fine-grained first chunks of wq (scalar queue) and xT block
            # (0,0) (sync queue), overlapping: the DMA path is cold at kernel
            # start, so small pieces land much earlier and the first
            # projection matmuls start sooner
            nc.sync.dma_start(wq_sb[:, :FPC], wq_d[:, :FPC])
            nc.sync.dma_start(xT_sb[:, :512], xT_d[:, :512])
            nc.sync.dma_start(wq_sb[:, FPC : 2 * FPC], wq_d[:, FPC : 2 * FPC])
            nc.sync.dma_start(xT_sb[:, 512:1024], xT_d[:, 512:1024])
            nc.sync.dma_start(wq_sb[:, 2 * FPC : 4 * FPC], wq_d[:, 2 * FPC : 4 * FPC])
            nc.sync.dma_start(xT_sb[:, 1024:2048], xT_d[:, 1024:2048])
            nc.sync.dma_start(wq_sb[:, 4 * FPC :], wq_d[:, 4 * FPC :])
            nc.sync.dma_start(xT_sb[:, 2048 : KC * 512], xT_d[:, 2048 : KC * 512])

            def xt_load(b, n):
                blk = (NT * b + n) * (KC * 512)
                nc.sync.dma_start(
                    xT_sb[:, blk : blk + KC * 512], xT_d[:, blk : blk + KC * 512]
                )
            nc.sync.dma_start(wk_sb[:], wk_d[:])
            nc.sync.dma_start(wv_sb[:], wv_d[:])
            nc.scalar.dma_start(C_sb[:], C_d[:])
            nc.scalar.dma_start(S_sb[:], S_d[:])
            nc.scalar.dma_start(id_sb[:], id_d[:])
            nc.scalar.dma_start(mask2_sb[:], mask2_d[:])
            nc.scalar.dma_start(sel_sb[:], sel_d[:])
            xt_load(0, 1)
            xt_load(0, 2)
            xt_load(0, 3)
            nc.sync.dma_start(wo_sb[:], wo_d[:])
            for n in range(NT):
                xt_load(1, n)

            # 4 AllToAll groups: group g carries global token chunks 8g+o to rank o
            a2a_in = [dram.tile([D, 128], BF16, name=f"a2ain{g}") for g in range(4)]
            a2a_out = [dram.tile([D, 128], BF16, name=f"a2aout{g}") for g in range(4)]

            # ================= filler queue machinery =================
            FQ = deque()          # items: (cost, label, fn)
            emitted = set()

            def push(units):
                FQ.extend(units)

            def _pop_one():
                cost, label, fn = FQ.popleft()
                fn()
                if label is not None:
                    emitted.add(label)
                return cost

            def drain(budget):
                while FQ and budget > 0:
                    budget -= _pop_one()
                # emit trailing zero-cost items (rope chains etc.) promptly
                while FQ and FQ[0][0] == 0:
                    _pop_one()

            def drain_until(lbl):
                if lbl in emitted:
                    return
                while FQ:
                    _, label, _ = FQ[0]
                    _pop_one()
                    if label == lbl:
                        return
                raise RuntimeError(f"label {lbl} not found in filler queue")

            def drain_all():
                while FQ:
                    _pop_one()

            # ================= projection tiles =================
            def ptile_units(b, n):
                """3 projection chains (q,k,v) + rope + V transpose for token
                block (b,n), as interleavable units."""
                st = {}
                blk = (NT * b + n) * (KC * 512)

                def mm(which, w_sb, kc):
                    def f():
                        if kc == 0:
                            st[which] = ps.tile(
                                [128, 512], F32, tag="proj", bufs=2,
                                name=f"pp{which}{b}{n}",
                            )
                        nc.tensor.matmul(
                            st[which][:],
                            w_sb[:, kc * FPC : (kc + 1) * FPC],
                            xT_sb[:, blk + 512 * kc : blk + 512 * kc + 512],
                            start=(kc == 0),
                            stop=(kc == KC - 1),
                        )
                    return (1.0, None, f)

                def rope(which, dst_sb):
                    def f():
                        pp = st[which]
                        swp = sb.tile([128, 512], F32, tag="swp", bufs=3,
                                      name=f"swp{which}{b}{n}")
                        nc.vector.stream_shuffle(swp[:], pp[:], swap16)
                        t1 = sb.tile([128, 512], BF16, tag="t1", bufs=3,
                                     name=f"t1{which}{b}{n}")
                        nc.vector.tensor_mul(t1[:], pp[:], C_sb[:, 512 * n : 512 * n + 512])
                        t2 = sb.tile([128, 512], BF16, tag="t2", bufs=3,
                                     name=f"t2{which}{b}{n}")
                        nc.vector.tensor_mul(t2[:], swp[:], S_sb[:, 512 * n : 512 * n + 512])
                        nc.vector.tensor_add(
                            dst_sb[:, b * T + 512 * n : b * T + 512 * n + 512],
                            t1[:], t2[:],
                        )
                    return (0.0, None, f)

                def vtt_copy():
                    def f():
                        vtt = sb.tile([128, 512], BF16, tag="vtt", bufs=2,
                                      name=f"vtt{b}{n}")
                        nc.vector.tensor_copy(vtt[:], st["v"][:])
                        st["vtt"] = vtt
                    return (0.0, None, f)

                def tp_i(i):
                    def f():
                        g = VG * ((T // 128) * b + 4 * n + i)
                        tp = ps.tile([128, 128], BF16, tag="proj", bufs=2,
                                     name=f"tp{b}{n}{i}")
                        nc.tensor.matmul(
                            tp[:],
                            st["vtt"][:, 128 * i : 128 * i + 128],
                            id_sb[:],
                            is_transpose=True,
                            start=True,
                            stop=True,
                        )
                        for h in range(2):
                            nc.vector.tensor_copy(
                                v1_sb[:, g + 128 * h : g + 128 * h + 64],
                                tp[:, 64 * h : 64 * h + 64],
                            )
                    return (0.5, None, f)

                units = []
                for kc in range(KC):
                    units.append(mm("q", wq_sb, kc))
                units.append(rope("q", qrot_sb))
                for kc in range(KC):
                    units.append(mm("k", wk_sb, kc))
                units.append(rope("k", krot_sb))
                for kc in range(KC):
                    units.append(mm("v", wv_sb, kc))
                units.append(vtt_copy())
                for i in range(4):
                    units.append(tp_i(i))
                # end marker (zero-cost no-op)
                units.append((0.0, ("Pend", b, n), lambda: None))
                return units

            # ================= attention =================
            def attn_tile(b, j):
                """Both heads for (batch b, q-tile j), chunk-granular with
                filler interleave. Returns [65,512] f32 SBUF copies of the
                two O accumulators (row 64 = softmax sums)."""
                drain_until(("Pend", b, j))
                ops = [
                    ps.tile([65, 512], F32, tag="opsum", bufs=2, name=f"op{b}{h}{j}")
                    for h in range(2)
                ]
                nch = 4 * j + 4

                def qk_exp(c):
                    diag = c - 4 * j
                    lo = 128 * diag if diag >= 0 else 0
                    sp = ps.tile([128, 1024], F32, tag="spsum", bufs=2,
                                 name=f"sp{b}{j}{c}")
                    spv = sp[:].rearrange("p (h t) -> p h t", h=2)
                    for h in range(2):
                        nc.tensor.matmul(
                            sp[:, 512 * h + lo : 512 * h + 512],
                            krot_sb[64 * h : 64 * h + 64,
                                    b * T + 128 * c : b * T + 128 * c + 128],
                            qrot_sb[64 * h : 64 * h + 64,
                                    b * T + 512 * j + lo : b * T + 512 * j + 512],
                            start=True,
                            stop=True,
                        )
                    pt = sb.tile([128, 1024], BF16, tag="pt", bufs=4,
                                 name=f"pt{b}{j}{c}")
                    ptv = pt[:].rearrange("p (h t) -> p h t", h=2)
                    nc.scalar.activation(
                        ptv[:, :, lo:512], spv[:, :, lo:512], AF.Exp, scale=0.125
                    )
                    if diag >= 0:
                        # zero the upper triangle post-exp (GpSimd: keeps DVE
                        # free for rope and keeps ACT pure-exp)
                        nc.gpsimd.tensor_mul(
                            ptv[:, :, lo : lo + 128], ptv[:, :, lo : lo + 128],
                            mask2_sb[:].rearrange("p (h t) -> p h t", h=2),
                        )
                    return pt

                def pv(c, pt):
                    diag = c - 4 * j
                    lo = 128 * diag if diag >= 0 else 0
                    g = VG * ((T // 128) * b + c)
                    for h in range(2):
                        nc.tensor.matmul(
                            ops[h][:, lo:512],
                            v1_sb[:, g + 128 * h : g + 128 * h + 65],
                            pt[:, 512 * h + lo : 512 * h + 512],
                            start=(c == 0),
                            stop=(c == nch - 1),
                        )

                prev = None
                for c in range(nch):
                    cur = qk_exp(c)
                    if prev is not None:
                        pv(c - 1, prev)
                    # interleave filler so the PE never head-blocks on exp
                    drain(2.5 if c < 4 * j else 1.5)
                    prev = cur
                pv(nch - 1, prev)
                o65s = []
                for h in range(2):
                    o65 = sb.tile([65, 512], F32, tag="o65", bufs=4,
                                  name=f"o65{b}{h}{j}")
                    nc.vector.tensor_copy(o65[:], ops[h][:])
                    o65s.append(o65)
                return o65s

            # ================= epilogue + collective =================
            at_tiles = {}

            def epi_pre(b, jpair, o65_by_j, sums_on_sync=False):
                """Per-pair normalization prefix: gather the 4 sum rows, one
                fast reciprocal, cast to bf16. All DVE/DMA - off the PE path.
                Mid-kernel pairs gather on the scalar queue (the sync queue
                carries at-loads whose collective waits would head-block
                these); the tail pair uses sync (no exp drain to wait out)."""
                units_meta = []
                for j in jpair:
                    for h in range(2):
                        units_meta.append((j, h, o65_by_j[j][h]))
                q = nc.sync if sums_on_sync else nc.scalar
                sums = sb.tile([4, 512], F32, tag="sums", bufs=2,
                               name=f"sums{b}{jpair[0]}")
                for r, (j, h, o65) in enumerate(units_meta):
                    q.dma_start(sums[r : r + 1, :], o65[64:65, :])
                rec4 = sb.tile([4, 512], F32, tag="rec4", bufs=2,
                               name=f"rec4{b}{jpair[0]}")
                nc.vector.reciprocal_approx_fast(out=rec4[:], in_=sums[:])
                recb4 = sb.tile([4, 512], BF16, tag="recb4", bufs=2,
                                name=f"recb4{b}{jpair[0]}")
                nc.vector.tensor_copy(recb4[:], rec4[:])
                return units_meta, recb4

            def epi_units(b, jpair, g, units_meta, recb4):
                """sel-matmul + normalize + stage, one unit per (j,h); then the
                collective trigger and the a2a-output loads."""
                units = []

                def unit(r, j, h, o65):
                    def f():
                        bps = ps.tile([64, 512], F32, tag="proj", bufs=2,
                                      name=f"bps{b}{j}{h}")
                        nc.tensor.matmul(
                            bps[:], sel_sb[:, DH * r : DH * r + DH], recb4[:],
                            start=True, stop=True,
                        )
                        onr = sb.tile([64, 512], BF16, tag="onr", bufs=4,
                                      name=f"onr{b}{j}{h}")
                        nc.vector.tensor_mul(onr[:], o65[0:64, :], bps[:])
                        # one 3D-AP staging DMA (partition dim first): 4
                        # token-chunks -> consecutive ranks o0..o0+3 of group g
                        o0 = 4 * (j % 2)
                        dst = a2a_in[g][:].rearrange(
                            "(o hh p) t -> hh p o t", hh=2, p=64
                        )[h, :, o0 : o0 + 4]
                        nc.scalar.dma_start(
                            dst, onr[:].rearrange("p (i t) -> p i t", i=4)
                        )
                    return (1.0, None, f)

                for r, (j, h, o65) in enumerate(units_meta):
                    units.append(unit(r, j, h, o65))

                def cc():
                    nc.gpsimd.collective_compute(
                        "AllToAll",
                        mybir.AluOpType.bypass,
                        replica_groups=[list(range(NCORES))],
                        ins=[a2a_in[g].opt()],
                        outs=[a2a_out[g].opt()],
                    )
                units.append((0.0, ("cc", g), cc))
                return units

            def load_at(g, nsplit=2):
                """Must be emitted AFTER collective g completes but BEFORE
                collective g+1's trigger pops - otherwise the DMA's wait
                resolves against the later collective and head-blocks the
                sync queue for tens of us. nsplit=8 (tail) lets final-matmul
                kc start as soon as chunk kc lands."""
                at = sb.tile([128, KC * 128], BF16, tag="at", bufs=4, name=f"at{g}")
                step = KC // nsplit
                for i in range(nsplit):
                    nc.sync.dma_start(
                        at[:].rearrange("p (o t) -> p o t", o=KC)[
                            :, step * i : step * i + step
                        ],
                        a2a_out[g][:].rearrange("(o p) t -> p o t", p=128)[
                            :, step * i : step * i + step
                        ],
                    )
                at_tiles[g] = at

            # ================= output projection =================
            def final_units(g, nhs=(0, 1)):
                units = []
                st = {}

                def mm(nh, kc):
                    def f():
                        if kc == 0:
                            st[nh] = ps.tile([128, 512], F32, tag="proj", bufs=2,
                                             name=f"fp{g}{nh}")
                        nc.tensor.matmul(
                            st[nh][:],
                            at_tiles[g][:, 128 * kc : 128 * kc + 128],
                            wo_sb[:, kc * D + 512 * nh : kc * D + 512 * nh + 512],
                            start=(kc == 0),
                            stop=(kc == KC - 1),
                        )
                    return (1.0, None, f)

                def store(nh):
                    def f():
                        fo = sb.tile([128, 512], F32, tag="fo", bufs=2,
                                     name=f"fo{g}{nh}")
                        nc.vector.tensor_copy(fo[:], st[nh][:])
                        nc.sync.dma_start(
                            out_d[128 * g : 128 * g + 128,
                                  512 * nh : 512 * nh + 512],
                            fo[:],
                        )
                    return (0.0, None, f)

                for nh in nhs:
                    for kc in range(KC):
                        units.append(mm(nh, kc))
                    units.append(store(nh))
                return units

            # ================= schedule =================
            o0, o1 = {}, {}
            push(ptile_units(0, 0))
            push(ptile_units(0, 1))
            drain_all()                       # dense startup
            push(ptile_units(0, 2))
            o0[0] = attn_tile(0, 0)
            push(ptile_units(0, 3))
            o0[1] = attn_tile(0, 1)
            m, r = epi_pre(0, (0, 1), o0)
            push(epi_units(0, (0, 1), 0, m, r))
            push(ptile_units(1, 0))
            o0[2] = attn_tile(0, 2)
            push(ptile_units(1, 1))
            o0[3] = attn_tile(0, 3)
            m, r = epi_pre(0, (2, 3), o0)
            load_at(0)                        # after cc0 done, before cc1 trigger
            push(epi_units(0, (2, 3), 1, m, r))
            push(ptile_units(1, 2))
            o1[0] = attn_tile(1, 0)
            o1[1] = attn_tile(1, 1)
            m, r = epi_pre(1, (0, 1), o1)
            load_at(1)                        # after cc1 done, before cc2 trigger
            push(epi_units(1, (0, 1), 2, m, r))
            push(ptile_units(1, 3))
            o1[2] = attn_tile(1, 2)
            push(final_units(0))
            o1[3] = attn_tile(1, 3)
            # last epilogue: start its DVE chain now, cover its latency with
            # leftover fillers + F1, then cover cc3's execution with F2
            m, r = epi_pre(1, (2, 3), o1)
            load_at(2)                        # after cc2 done, before cc3 trigger
            push(final_units(1))
            push(epi_units(1, (2, 3), 3, m, r))
            drain_all()                       # leftovers + F1, then sel/stage + cc3
            push(final_units(2))
            drain_all()
            load_at(3, nsplit=8)
            push(final_units(3))
            drain_all()

    nc.compile()
    return nc


def _get_compiled():
    global _COMPILED
    if _COMPILED is None:
        _COMPILED = _build()
    return _COMPILED


def _kc_flat(a):
    """[D, N] -> [128, KC*N] with row-chunk kc at column block kc."""
    Dn, N = a.shape
    assert Dn == KC * 128
    return np.ascontiguousarray(
        a.reshape(KC, 128, N).transpose(1, 0, 2).reshape(128, KC * N)
    )


def _prep_in_maps(embedding_word, wq, wk, wv, wo):
    bf = ml_dtypes.bfloat16
    x = np.asarray(embedding_word, np.float32).reshape(TOK, D)

    # xT in the exact SBUF layout: token-block (b,n) major, kc-chunk inside
    x5 = x.reshape(B, NT, 512, KC, 128)            # [b, n, t, kc, p]
    xTf = np.ascontiguousarray(
        x5.transpose(4, 0, 1, 3, 2).reshape(128, KC * TOK)
    ).astype(bf)

    wof = _kc_flat(np.ascontiguousarray(np.asarray(wo, np.float32).T)).astype(bf)

    # within-head row permutation: 16 re rows then 16 im rows per 32-row quadrant
    perm64 = [
        (2 * (16 * q + r) if r < 16 else 2 * (16 * q + (r - 16)) + 1)
        for q in range(2)
        for r in range(32)
    ]
    perm64 = np.asarray(perm64)

    freqs = 1.0 / (10000.0 ** (np.arange(0, DH, 2, dtype=np.float64) / DH))  # [32]
    ang = np.arange(T, dtype=np.float64)[:, None] * freqs[None, :]  # [T, 32]
    cos_t, sin_t = np.cos(ang), np.sin(ang)
    rows = np.arange(128)
    wh = rows % 64
    qd = wh // 32
    r32 = wh % 32
    dmap = 16 * qd + (r32 % 16)
    sign = np.where(r32 < 16, -1.0, 1.0)
    C = np.ascontiguousarray(cos_t[:, dmap].T).astype(bf)  # [128, T]
    S = np.ascontiguousarray((sin_t[:, dmap] * sign[None, :]).T).astype(bf)

    rr = np.arange(128)[:, None]
    cc = np.arange(128)[None, :]
    mask = np.where(cc >= rr, 1.0, 0.0).astype(np.float32)
    mask2 = np.ascontiguousarray(np.concatenate([mask, mask], axis=1)).astype(bf)
    ident = np.eye(128, dtype=np.float32).astype(bf)
    sel = np.zeros((4, 4 * DH), np.float32)
    for r in range(4):
        sel[r, DH * r : DH * r + DH] = 1.0
    sel = sel.astype(bf)

    wqf = np.asarray(wq, np.float32)
    wkf = np.asarray(wk, np.float32)
    wvf = np.asarray(wv, np.float32)

    in_maps = []
    for c in range(NCORES):
        rows_c = slice(FPC * c, FPC * c + FPC)
        wq_c = wqf[rows_c].reshape(HPC, DH, D)[:, perm64, :].reshape(FPC, D)
        wk_c = wkf[rows_c].reshape(HPC, DH, D)[:, perm64, :].reshape(FPC, D)
        wv_c = wvf[rows_c]
        in_maps.append(
            {
                "xTf": xTf,
                "wqf": _kc_flat(np.ascontiguousarray(wq_c.T)).astype(bf),
                "wkf": _kc_flat(np.ascontiguousarray(wk_c.T)).astype(bf),
                "wvf": _kc_flat(np.ascontiguousarray(wv_c.T)).astype(bf),
                "wof": wof,
                "cosC": C,
                "sinS": S,
                "mask2": mask2,
                "ident": ident,
                "sel": sel,
            }
        )
    return in_maps


def _unshard(core_outs):
    """core_outs[c] is [TPC, D] covering token chunks {c, 8+c, 16+c, 24+c}
    (row-blocks g=0..3). Interleave back to [B, T, D]."""
    a = np.stack(core_outs, axis=0)  # [8, TPC, D]
    a = a.reshape(NCORES, 4, 128, D).transpose(1, 0, 2, 3).reshape(TOK, D)
    return np.ascontiguousarray(a.reshape(B, T, D).astype(np.float32))


def kernel(embedding_word, wq, wk, wv, wo):
    nc = _get_compiled()
    in_maps = _prep_in_maps(embedding_word, wq, wk, wv, wo)
    res = bass_utils.run_bass_kernel_spmd(nc, in_maps, core_ids=list(range(NCORES)))
    return _unshard([res.results[c]["out"] for c in range(NCORES)])


# revision 26
# speedup vs baseline: 1.0527x; 1.0051x over previous
"""Multi-head causal attention with RoPE on 8 TRN2 NeuronCores.

Problem: B=2, T=2048, D=1024, H=16 heads (dh=64), fp32 I/O.
  q/k/v = x @ w{q,k,v}.T ; RoPE(q,k) ; causal softmax((q k^T)/sqrt(dh)) @ v ;
  out = concat_heads @ wo.T

Sharding (8 cores): head-parallel compute, token-striped output. Core c owns
heads {2c, 2c+1} for both batches; four AllToAll collectives redistribute
attention outputs so core c ends up with all 1024 features for its four
128-token chunks {c, 8+c, 16+c, 24+c}; it then applies the output projection.

v2 design notes (driven by the baseline's perfetto profile):
 - Engine-queue discipline: ACT runs ONLY the exp stream (the o65/vtt/fo
   copies moved to DVE, causal-mask muls to GpSimd, weight DMAs to the
   GpSimd DGE queue). The baseline's ACT queue carried 43us of copies+DMA
   which head-blocked the PE whenever it waited for an S-psum bank drain.
 - Chunk-granular software pipeline: projection / output-projection matmuls
   are interleaved 1-2 per attention chunk from a FIFO filler queue, so the
   PE always has independent work while ACT chews exp (exp ~1.15us/chunk vs
   ~0.8us of attention matmul per chunk). This also keeps the PE out of the
   HAM K=4/8 half-clock re-throttle state (idle >3.4us triggers it).
 - All input DMAs are flat [128, N] copies: the host pre-arranges xT and the
   weights into the exact SBUF layout (the baseline's (k p) c -> p k c
   rearranged DMAs issued 256B-element descriptor storms; first matmul was
   at t=16us, now ~9us).
 - reciprocal -> reciprocal_approx_fast (13.3us -> ~2.7us of DVE).
 - A2A staging is one 4D-AP DMA per (j,h) unit instead of 4 2D DMAs; a2a
   output is loaded as 8 flat [128,128] chunks consumed directly as lhsT.
 - Tail: the last a2a is covered by held-back final-projection matmuls for
   groups 1-2 so the PE stays busy (and warm) during the collective.

Layout notes (unchanged from baseline):
 - Host pre-transposes x -> xT [D, B*T] so projections produce q^T/k^T
   directly; RoPE pairs de-interleaved on the host via wq/wk row permutation
   (16 re rows then 16 im rows per 32-row quadrant); partner swap is a DVE
   stream_shuffle.
 - A ones column is appended to each V chunk so the softmax denominator
   falls out of the PV matmul (row 64 of the accumulator).
 - The two heads' K=64 QK matmuls use PE row-groups 0-63 / 64-127 and run
   concurrently in the systolic array.
 - Softmax skips the running-max: |scores|/8 < ~6 for unit-variance inputs.
"""

from collections import deque

import numpy as np
import ml_dtypes

import concourse.bacc as bacc
import concourse.tile as tile
import concourse.mybir as mybir
from concourse import bass_utils

BF16 = mybir.dt.bfloat16
F32 = mybir.dt.float32
AF = mybir.ActivationFunctionType

NCORES = 8
B, T, D, H = 2, 2048, 1024, 16
DH = D // H          # 64
HPC = H // NCORES    # 2 heads per core
FPC = DH * HPC       # 128 features per core
TOK = B * T          # 4096
TPC = TOK // NCORES  # 512 tokens per core (output shard)
KC = D // 128        # 8 contraction chunks
NT = T // 512        # 4 query tiles of 512 per batch
VG = 256             # cols per v-group: [v_h0(64) | 1 | pad | v_h1(64) | 1 | pad]

_COMPILED = None


def _build():
    nc = bacc.Bacc("TRN2", target_bir_lowering=False, debug=False, num_devices=NCORES)

    # All inputs are host-prearranged into their exact SBUF layouts (flat DMAs).
    xT_d = nc.dram_tensor("xTf", [128, KC * TOK], BF16, kind="ExternalInput")
    wq_d = nc.dram_tensor("wqf", [128, KC * FPC], BF16, kind="ExternalInput")
    wk_d = nc.dram_tensor("wkf", [128, KC * FPC], BF16, kind="ExternalInput")
    wv_d = nc.dram_tensor("wvf", [128, KC * FPC], BF16, kind="ExternalInput")
    wo_d = nc.dram_tensor("wof", [128, KC * D], BF16, kind="ExternalInput")
    C_d = nc.dram_tensor("cosC", [128, T], BF16, kind="ExternalInput")
    S_d = nc.dram_tensor("sinS", [128, T], BF16, kind="ExternalInput")
    mask2_d = nc.dram_tensor("mask2", [128, 256], BF16, kind="ExternalInput")
    id_d = nc.dram_tensor("ident", [128, 128], BF16, kind="ExternalInput")
    sel_d = nc.dram_tensor("sel", [4, 4 * DH], BF16, kind="ExternalInput")
    out_d = nc.dram_tensor("out", [TPC, D], F32, kind="ExternalOutput")

    swap16 = list(range(16, 32)) + list(range(16))

    with tile.TileContext(nc) as tc:
        with (
            tc.tile_pool(name="sb", bufs=1) as sb,
            tc.tile_pool(name="ps", bufs=1, space="PSUM") as ps,
            tc.tile_pool(name="dram", bufs=1, space="DRAM") as dram,
        ):
            # ---- persistent SBUF residents ----
            wq_sb = sb.tile([128, KC * FPC], BF16)
            wk_sb = sb.tile([128, KC * FPC], BF16)
            wv_sb = sb.tile([128, KC * FPC], BF16)
            wo_sb = sb.tile([128, KC * D], BF16)
            C_sb = sb.tile([128, T], BF16)
            S_sb = sb.tile([128, T], BF16)
            mask2_sb = sb.tile([128, 256], BF16)
            id_sb = sb.tile([128, 128], BF16)
            sel_sb = sb.tile([4, 4 * DH], BF16)
            xT_sb = sb.tile([128, KC * TOK], BF16)
            qrot_sb = sb.tile([128, TOK], BF16)
            krot_sb = sb.tile([128, TOK], BF16)
            v1_sb = sb.tile([128, B * (T // 128) * VG], BF16)

            # ones column (softmax denominator) in each 128-col v slot
            nc.gpsimd.memset(
                v1_sb[:].rearrange("p (g c) -> p g c", c=128)[:, :, 64:65], 1.0
            )
            # warm the exp table set while DMAs stream (first real exp is ~25us in)
            tblw = sb.tile([1, 1], F32)
            nc.scalar.activation(tblw[:], v1_sb[0:1, 64:65], AF.Exp)

            # ---- input DMAs, all flat copies on HWDGE queues (gpsimd DMAs
            # are software-DGE at ~24GB/s - never put loads there).
            # sync: wq + first xT block first (gates the first matmul), then
            # the rest of the stream. vector: rope/mask constants (DVE is
            # scalar queue is idle until the first exp at ~25us).
            # fine-grained first chunks of wq (scalar queue) and xT block
            # (0,0) (sync queue), overlapping: the DMA path is cold at kernel
            # start, so small pieces land much earlier and the first
            # projection matmuls start sooner
            nc.sync.dma_start(wq_sb[:, :FPC], wq_d[:, :FPC])
            nc.sync.dma_start(xT_sb[:, :512], xT_d[:, :512])
            nc.sync.dma_start(wq_sb[:, FPC : 2 * FPC], wq_d[:, FPC : 2 * FPC])
            nc.sync.dma_start(xT_sb[:, 512:1024], xT_d[:, 512:1024])
            nc.sync.dma_start(wq_sb[:, 2 * FPC : 4 * FPC], wq_d[:, 2 * FPC : 4 * FPC])
            nc.sync.dma_start(xT_sb[:, 1024:2048], xT_d[:, 1024:2048])
            nc.sync.dma_start(wq_sb[:, 4 * FPC :], wq_d[:, 4 * FPC :])
            nc.sync.dma_start(xT_sb[:, 2048 : KC * 512], xT_d[:, 2048 : KC * 512])

            def xt_load(b, n):
                blk = (NT * b + n) * (KC * 512)
                nc.sync.dma_start(
                    xT_sb[:, blk : blk + KC * 512], xT_d[:, blk : blk + KC * 512]
                )
            nc.sync.dma_start(wk_sb[:], wk_d[:])
            nc.sync.dma_start(wv_sb[:], wv_d[:])
            nc.scalar.dma_start(C_sb[:], C_d[:])
            nc.scalar.dma_start(S_sb[:], S_d[:])
            nc.scalar.dma_start(id_sb[:], id_d[:])
            nc.scalar.dma_start(mask2_sb[:], mask2_d[:])
            nc.scalar.dma_start(sel_sb[:], sel_d[:])
            xt_load(0, 1)
            xt_load(0, 2)
            xt_load(0, 3)
            nc.sync.dma_start(wo_sb[:], wo_d[:])
            for n in range(NT):
                xt_load(1, n)

            # 4 AllToAll groups: group g carries global token chunks 8g+o to rank o
            a2a_in = [dram.tile([D, 128], BF16, name=f"a2ain{g}") for g in range(4)]
            a2a_out = [dram.tile([D, 128], BF16, name=f"a2aout{g}") for g in range(4)]

            # ================= filler queue machinery =================
            FQ = deque()          # items: (cost, label, fn)
            emitted = set()

            def push(units):
                FQ.extend(units)

            def _pop_one():
                cost, label, fn = FQ.popleft()
                fn()
                if label is not None:
                    emitted.add(label)
                return cost

            def drain(budget):
                while FQ and budget > 0:
                    budget -= _pop_one()
                # emit trailing zero-cost items (rope chains etc.) promptly
                while FQ and FQ[0][0] == 0:
                    _pop_one()

            def drain_until(lbl):
                if lbl in emitted:
                    return
                while FQ:
                    _, label, _ = FQ[0]
                    _pop_one()
                    if label == lbl:
                        return
                raise RuntimeError(f"label {lbl} not found in filler queue")

            def drain_all():
                while FQ:
                    _pop_one()

            # ================= projection tiles =================
            def ptile_units(b, n):
                """3 projection chains (q,k,v) + rope + V transpose for token
                block (b,n), as interleavable units."""
                st = {}
                blk = (NT * b + n) * (KC * 512)

                def mm(which, w_sb, kc):
                    def f():
                        if kc == 0:
                            st[which] = ps.tile(
                                [128, 512], F32, tag="proj", bufs=2,
                                name=f"pp{which}{b}{n}",
                            )
                        nc.tensor.matmul(
                            st[which][:],
                            w_sb[:, kc * FPC : (kc + 1) * FPC],
                            xT_sb[:, blk + 512 * kc : blk + 512 * kc + 512],
                            start=(kc == 0),
                            stop=(kc == KC - 1),
                        )
                    return (1.0, None, f)

                def rope(which, dst_sb):
                    def f():
                        pp = st[which]
                        swp = sb.tile([128, 512], F32, tag="swp", bufs=3,
                                      name=f"swp{which}{b}{n}")
                        nc.vector.stream_shuffle(swp[:], pp[:], swap16)
                        t1 = sb.tile([128, 512], BF16, tag="t1", bufs=3,
                                     name=f"t1{which}{b}{n}")
                        nc.vector.tensor_mul(t1[:], pp[:], C_sb[:, 512 * n : 512 * n + 512])
                        t2 = sb.tile([128, 512], BF16, tag="t2", bufs=3,
                                     name=f"t2{which}{b}{n}")
                        nc.vector.tensor_mul(t2[:], swp[:], S_sb[:, 512 * n : 512 * n + 512])
                        nc.vector.tensor_add(
                            dst_sb[:, b * T + 512 * n : b * T + 512 * n + 512],
                            t1[:], t2[:],
                        )
                    return (0.0, None, f)

                def vtt_copy():
                    def f():
                        vtt = sb.tile([128, 512], BF16, tag="vtt", bufs=2,
                                      name=f"vtt{b}{n}")
                        nc.vector.tensor_copy(vtt[:], st["v"][:])
                        st["vtt"] = vtt
                    return (0.0, None, f)

                def tp_i(i):
                    def f():
                        g = VG * ((T // 128) * b + 4 * n + i)
                        tp = ps.tile([128, 128], BF16, tag="proj", bufs=2,
                                     name=f"tp{b}{n}{i}")
                        nc.tensor.matmul(
                            tp[:],
                            st["vtt"][:, 128 * i : 128 * i + 128],
                            id_sb[:],
                            is_transpose=True,
                            start=True,
                            stop=True,
                        )
                        for h in range(2):
                            nc.vector.tensor_copy(
                                v1_sb[:, g + 128 * h : g + 128 * h + 64],
                                tp[:, 64 * h : 64 * h + 64],
                            )
                    return (0.5, None, f)

                units = []
                for kc in range(KC):
                    units.append(mm("q", wq_sb, kc))
                units.append(rope("q", qrot_sb))
                for kc in range(KC):
                    units.append(mm("k", wk_sb, kc))
                units.append(rope("k", krot_sb))
                for kc in range(KC):
                    units.append(mm("v", wv_sb, kc))
                units.append(vtt_copy())
                for i in range(4):
                    units.append(tp_i(i))
                # end marker (zero-cost no-op)
                units.append((0.0, ("Pend", b, n), lambda: None))
                return units

            # ================= attention =================
            def attn_tile(b, j):
                """Both heads for (batch b, q-tile j), chunk-granular with
                filler interleave. Returns [65,512] f32 SBUF copies of the
                two O accumulators (row 64 = softmax sums)."""
                drain_until(("Pend", b, j))
                ops = [
                    ps.tile([65, 512], F32, tag="opsum", bufs=2, name=f"op{b}{h}{j}")
                    for h in range(2)
                ]
                nch = 4 * j + 4

                def qk_exp(c):
                    diag = c - 4 * j
                    lo = 128 * diag if diag >= 0 else 0
                    sp = ps.tile([128, 1024], F32, tag="spsum", bufs=2,
                                 name=f"sp{b}{j}{c}")
                    spv = sp[:].rearrange("p (h t) -> p h t", h=2)
                    for h in range(2):
                        nc.tensor.matmul(
                            sp[:, 512 * h + lo : 512 * h + 512],
                            krot_sb[64 * h : 64 * h + 64,
                                    b * T + 128 * c : b * T + 128 * c + 128],
                            qrot_sb[64 * h : 64 * h + 64,
                                    b * T + 512 * j + lo : b * T + 512 * j + 512],
                            start=True,
                            stop=True,
                        )
                    pt = sb.tile([128, 1024], BF16, tag="pt", bufs=4,
                                 name=f"pt{b}{j}{c}")
                    ptv = pt[:].rearrange("p (h t) -> p h t", h=2)
                    nc.scalar.activation(
                        ptv[:, :, lo:512], spv[:, :, lo:512], AF.Exp, scale=0.125
                    )
                    if diag >= 0:
                        # zero the upper triangle post-exp (GpSimd: keeps DVE
                        # free for rope and keeps ACT pure-exp)
                        nc.gpsimd.tensor_mul(
                            ptv[:, :, lo : lo + 128], ptv[:, :, lo : lo + 128],
                            mask2_sb[:].rearrange("p (h t) -> p h t", h=2),
                        )
                    return pt

                def pv(c, pt):
                    diag = c - 4 * j
                    lo = 128 * diag if diag >= 0 else 0
                    g = VG * ((T // 128) * b + c)
                    for h in range(2):
                        nc.tensor.matmul(
                            ops[h][:, lo:512],
                            v1_sb[:, g + 128 * h : g + 128 * h + 65],
                            pt[:, 512 * h + lo : 512 * h + 512],
                            start=(c == 0),
                            stop=(c == nch - 1),
                        )

                prev = None
                for c in range(nch):
                    cur = qk_exp(c)
                    if prev is not None:
                        pv(c - 1, prev)
                    # interleave filler so the PE never head-blocks on exp
                    drain(2.5 if c < 4 * j else 1.5)
                    prev = cur
                pv(nch - 1, prev)
                o65s = []
                for h in range(2):
                    o65 = sb.tile([65, 512], F32, tag="o65", bufs=4,
                                  name=f"o65{b}{h}{j}")
                    nc.vector.tensor_copy(o65[:], ops[h][:])
                    o65s.append(o65)
                return o65s

            # ================= epilogue + collective =================
            at_tiles = {}

            def epi_pre(b, jpair, o65_by_j, sums_on_sync=False):
                """Per-pair normalization prefix: gather the 4 sum rows, one
                fast reciprocal, cast to bf16. Mid-kernel pairs gather on the
                scalar queue (the sync queue carries at-loads whose collective
                waits could head-block these); the tail pair uses sync, which
                is idle there - no exp drain ahead of it."""
                units_meta = []
                for j in jpair:
                    for h in range(2):
                        units_meta.append((j, h, o65_by_j[j][h]))
                q = nc.sync if sums_on_sync else nc.scalar
                sums = sb.tile([4, 512], F32, tag="sums", bufs=2,
                               name=f"sums{b}{jpair[0]}")
                for r, (j, h, o65) in enumerate(units_meta):
                    q.dma_start(sums[r : r + 1, :], o65[64:65, :])
                rec4 = sb.tile([4, 512], F32, tag="rec4", bufs=2,
                               name=f"rec4{b}{jpair[0]}")
                nc.vector.reciprocal_approx_fast(out=rec4[:], in_=sums[:])
                recb4 = sb.tile([4, 512], BF16, tag="recb4", bufs=2,
                                name=f"recb4{b}{jpair[0]}")
                nc.vector.tensor_copy(recb4[:], rec4[:])
                return units_meta, recb4

            def epi_units(b, jpair, g, units_meta, recb4):
                """sel-matmul + normalize + stage, one unit per (j,h); then the
                collective trigger and the a2a-output loads."""
                units = []

                def unit(r, j, h, o65):
                    def f():
                        bps = ps.tile([64, 512], F32, tag="proj", bufs=2,
                                      name=f"bps{b}{j}{h}")
                        nc.tensor.matmul(
                            bps[:], sel_sb[:, DH * r : DH * r + DH], recb4[:],
                            start=True, stop=True,
                        )
                        onr = sb.tile([64, 512], BF16, tag="onr", bufs=4,
                                      name=f"onr{b}{j}{h}")
                        nc.vector.tensor_mul(onr[:], o65[0:64, :], bps[:])
                        # one 3D-AP staging DMA (partition dim first): 4
                        # token-chunks -> consecutive ranks o0..o0+3 of group g
                        o0 = 4 * (j % 2)
                        dst = a2a_in[g][:].rearrange(
                            "(o hh p) t -> hh p o t", hh=2, p=64
                        )[h, :, o0 : o0 + 4]
                        nc.scalar.dma_start(
                            dst, onr[:].rearrange("p (i t) -> p i t", i=4)
                        )
                    return (1.0, None, f)

                for r, (j, h, o65) in enumerate(units_meta):
                    units.append(unit(r, j, h, o65))

                def cc():
                    nc.gpsimd.collective_compute(
                        "AllToAll",
                        mybir.AluOpType.bypass,
                        replica_groups=[list(range(NCORES))],
                        ins=[a2a_in[g].opt()],
                        outs=[a2a_out[g].opt()],
                    )
                units.append((0.0, ("cc", g), cc))
                return units

            def load_at(g, nsplit=2):
                """Must be emitted AFTER collective g completes but BEFORE
                collective g+1's trigger pops - otherwise the DMA's wait
                resolves against the later collective and head-blocks the
                sync queue for tens of us. nsplit=8 (tail) lets final-matmul
                kc start as soon as chunk kc lands."""
                at = sb.tile([128, KC * 128], BF16, tag="at", bufs=4, name=f"at{g}")
                step = KC // nsplit
                for i in range(nsplit):
                    nc.sync.dma_start(
                        at[:].rearrange("p (o t) -> p o t", o=KC)[
                            :, step * i : step * i + step
                        ],
                        a2a_out[g][:].rearrange("(o p) t -> p o t", p=128)[
                            :, step * i : step * i + step
                        ],
                    )
                at_tiles[g] = at

            # ================= output projection =================
            def final_units(g, nhs=(0, 1)):
                units = []
                st = {}

                def mm(nh, kc):
                    def f():
                        if kc == 0:
                            st[nh] = ps.tile([128, 512], F32, tag="proj", bufs=2,
                                             name=f"fp{g}{nh}")
                        nc.tensor.matmul(
                            st[nh][:],
                            at_tiles[g][:, 128 * kc : 128 * kc + 128],
                            wo_sb[:, kc * D + 512 * nh : kc * D + 512 * nh + 512],
                            start=(kc == 0),
                            stop=(kc == KC - 1),
                        )
                    return (1.0, None, f)

                def store(nh):
                    def f():
                        fo = sb.tile([128, 512], F32, tag="fo", bufs=2,
                                     name=f"fo{g}{nh}")
                        nc.vector.tensor_copy(fo[:], st[nh][:])
                        nc.sync.dma_start(
                            out_d[128 * g : 128 * g + 128,
                                  512 * nh : 512 * nh + 512],
                            fo[:],
                        )
                    return (0.0, None, f)

                for nh in nhs:
                    for kc in range(KC):
                        units.append(mm(nh, kc))
                    units.append(store(nh))
                return units

            # ================= schedule =================
            o0, o1 = {}, {}
            push(ptile_units(0, 0))
            push(ptile_units(0, 1))
            drain_all()                       # dense startup
            push(ptile_units(0, 2))
            o0[0] = attn_tile(0, 0)
            push(ptile_units(0, 3))
            o0[1] = attn_tile(0, 1)
            m, r = epi_pre(0, (0, 1), o0)
            push(epi_units(0, (0, 1), 0, m, r))
            push(ptile_units(1, 0))
            o0[2] = attn_tile(0, 2)
            push(ptile_units(1, 1))
            o0[3] = attn_tile(0, 3)
            m, r = epi_pre(0, (2, 3), o0)
            load_at(0)                        # after cc0 done, before cc1 trigger
            push(epi_units(0, (2, 3), 1, m, r))
            push(ptile_units(1, 2))
            o1[0] = attn_tile(1, 0)
            o1[1] = attn_tile(1, 1)
            m, r = epi_pre(1, (0, 1), o1)
            load_at(1)                        # after cc1 done, before cc2 trigger
            push(epi_units(1, (0, 1), 2, m, r))
            push(ptile_units(1, 3))
            o1[2] = attn_tile(1, 2)
            push(final_units(0))
            o1[3] = attn_tile(1, 3)
            # last epilogue: start its DVE chain now, cover its latency with
            # leftover fillers + F1, then cover cc3's execution with F2
            m, r = epi_pre(1, (2, 3), o1, sums_on_sync=True)
            load_at(2)                        # after cc2 done, before cc3 trigger
            push(epi_units(1, (2, 3), 3, m, r))
            drain_all()                       # leftovers cover the chain; cc3 fires
            push(final_units(1))              # F1+F2 cover cc3's execution
            push(final_units(2))
            drain_all()
            load_at(3, nsplit=8)
            push(final_units(3))
            drain_all()

    nc.compile()
    return nc


def _get_compiled():
    global _COMPILED
    if _COMPILED is None:
        _COMPILED = _build()
    return _COMPILED


def _kc_flat(a):
    """[D, N] -> [128, KC*N] with row-chunk kc at column block kc."""
    Dn, N = a.shape
    assert Dn == KC * 128
    return np.ascontiguousarray(
        a.reshape(KC, 128, N).transpose(1, 0, 2).reshape(128, KC * N)
    )


def _prep_in_maps(embedding_word, wq, wk, wv, wo):
    bf = ml_dtypes.bfloat16
    x = np.asarray(embedding_word, np.float32).reshape(TOK, D)

    # xT in the exact SBUF layout: token-block (b,n) major, kc-chunk inside
    x5 = x.reshape(B, NT, 512, KC, 128)            # [b, n, t, kc, p]
    xTf = np.ascontiguousarray(
        x5.transpose(4, 0, 1, 3, 2).reshape(128, KC * TOK)
    ).astype(bf)

    wof = _kc_flat(np.ascontiguousarray(np.asarray(wo, np.float32).T)).astype(bf)

    # within-head row permutation: 16 re rows then 16 im rows per 32-row quadrant
    perm64 = [
        (2 * (16 * q + r) if r < 16 else 2 * (16 * q + (r - 16)) + 1)
        for q in range(2)
        for r in range(32)
    ]
    perm64 = np.asarray(perm64)

    freqs = 1.0 / (10000.0 ** (np.arange(0, DH, 2, dtype=np.float64) / DH))  # [32]
    ang = np.arange(T, dtype=np.float64)[:, None] * freqs[None, :]  # [T, 32]
    cos_t, sin_t = np.cos(ang), np.sin(ang)
    rows = np.arange(128)
    wh = rows % 64
    qd = wh // 32
    r32 = wh % 32
    dmap = 16 * qd + (r32 % 16)
    sign = np.where(r32 < 16, -1.0, 1.0)
    C = np.ascontiguousarray(cos_t[:, dmap].T).astype(bf)  # [128, T]
    S = np.ascontiguousarray((sin_t[:, dmap] * sign[None, :]).T).astype(bf)

    rr = np.arange(128)[:, None]
    cc = np.arange(128)[None, :]
    mask = np.where(cc >= rr, 1.0, 0.0).astype(np.float32)
    mask2 = np.ascontiguousarray(np.concatenate([mask, mask], axis=1)).astype(bf)
    ident = np.eye(128, dtype=np.float32).astype(bf)
    sel = np.zeros((4, 4 * DH), np.float32)
    for r in range(4):
        sel[r, DH * r : DH * r + DH] = 1.0
    sel = sel.astype(bf)

    wqf = np.asarray(wq, np.float32)
    wkf = np.asarray(wk, np.float32)
    wvf = np.asarray(wv, np.float32)

    in_maps = []
    for c in range(NCORES):
        rows_c = slice(FPC * c, FPC * c + FPC)
        wq_c = wqf[rows_c].reshape(HPC, DH, D)[:, perm64, :].reshape(FPC, D)
        wk_c = wkf[rows_c].reshape(HPC, DH, D)[:, perm64, :].reshape(FPC, D)
        wv_c = wvf[rows_c]
        in_maps.append(
            {
                "xTf": xTf,
                "wqf": _kc_flat(np.ascontiguousarray(wq_c.T)).astype(bf),
                "wkf": _kc_flat(np.ascontiguousarray(wk_c.T)).astype(bf),
                "wvf": _kc_flat(np.ascontiguousarray(wv_c.T)).astype(bf),
                "wof": wof,
                "cosC": C,
                "sinS": S,
                "mask2": mask2,
                "ident": ident,
                "sel": sel,
            }
        )
    return in_maps


def _unshard(core_outs):
    """core_outs[c] is [TPC, D] covering token chunks {c, 8+c, 16+c, 24+c}
    (row-blocks g=0..3). Interleave back to [B, T, D]."""
    a = np.stack(core_outs, axis=0)  # [8, TPC, D]
    a = a.reshape(NCORES, 4, 128, D).transpose(1, 0, 2, 3).reshape(TOK, D)
    return np.ascontiguousarray(a.reshape(B, T, D).astype(np.float32))


def kernel(embedding_word, wq, wk, wv, wo):
    nc = _get_compiled()
    in_maps = _prep_in_maps(embedding_word, wq, wk, wv, wo)
    res = bass_utils.run_bass_kernel_spmd(nc, in_maps, core_ids=list(range(NCORES)))
    return _unshard([res.results[c]["out"] for c in range(NCORES)])
